# revision 19
# baseline (speedup 1.0000x reference)
"""Trainium2 Bass kernel for nn_Encoder_88691074663154 (dense transformer encoder layer).

Strategy: sequence-parallel over 8 NeuronCores. Each core owns L/8 = 256
sequence positions (x B=4 batches = 1024 tokens). QKV is computed locally for
the token shard; K and V shards are AllGathered so every core attends its
queries against all 2048 keys. Everything downstream (out-proj, layernorms,
FFN) is token-local. Activations stay feature-major (features on SBUF
partitions) end to end, which makes every matmul transpose-free and every
bias/LN-gain a per-partition scalar; the host transposes shard outputs back.

Softmax: scores are computed transposed (keys on partitions, queries free);
exp on ScalarE; the softmax denominator comes for free from an extra ones
column appended to V (row 64 of the AV accumulator); normalization is a
per-head K=1 replicate-matmul + elementwise multiply. LayerNorm mean/var use
ones-column matmuls (partition reduction on the PE) + K=1 replicate matmuls.

Matmuls run in fp32r (full PE rate, ~1e-4 worst-case rounding on
compute-produced operands, bit-accurate on DMA-fed ones).
"""
import os
import sys

sys.path.insert(0, "/opt/trn_rl_repo")

_STAGE = os.environ.get("K_STAGE", "full")  # q | qa | full

import numpy as np

import concourse.bacc as bacc
import concourse.mybir as mybir
import concourse.tile as tile
from concourse.bass_utils import run_bass_kernel_spmd

F32 = mybir.dt.float32
F32R = mybir.dt.float32r
AF = mybir.ActivationFunctionType
OP = mybir.AluOpType

L, B, E, H, HD, HID = 2048, 4, 512, 8, 64, 2048
NCORES = 8
LC = L // NCORES          # 256 sequence positions per core
TOK = B * LC              # 1024 tokens per core (batch-major: j = b*LC + l_local)
EC = E // 128             # 4 feature chunks
HIDC = HID // 128         # 16 hidden chunks
QB = LC                   # queries per (batch) on this core = 256

_BUILD_CACHE = {}


def build_encoder():
    global _STAGE
    _STAGE = os.environ.get("K_STAGE", "full")
    if "nc" in _BUILD_CACHE:
        return _BUILD_CACHE["nc"]
    nc = bacc.Bacc(None, num_devices=NCORES)

    # ---- DRAM parameters (per core) ----
    xT_in = nc.declare_dram_parameter("xT", [E, TOK], F32, isOutput=False)
    peT_in = nc.declare_dram_parameter("peT", [E, TOK], F32, isOutput=False)
    wqkT_in = nc.declare_dram_parameter("wqkT", [E, 2 * E], F32, isOutput=False)
    wvT_in = nc.declare_dram_parameter("wvT", [E, E], F32, isOutput=False)
    woT_in = nc.declare_dram_parameter("woT", [E, E], F32, isOutput=False)
    w1T_in = nc.declare_dram_parameter("w1T", [E, HID], F32, isOutput=False)
    w2T_in = nc.declare_dram_parameter("w2T", [HID, E], F32, isOutput=False)
    bqk_in = nc.declare_dram_parameter("bqk2d", [128, 8], F32, isOutput=False)
    bvr_in = nc.declare_dram_parameter("bv_rep", [128, E], F32, isOutput=False)
    bo_in = nc.declare_dram_parameter("bo2d", [128, EC], F32, isOutput=False)
    b1_in = nc.declare_dram_parameter("b1_2d", [128, HIDC], F32, isOutput=False)
    b2_in = nc.declare_dram_parameter("b2_2d", [128, EC], F32, isOutput=False)
    g_in = nc.declare_dram_parameter("g2d", [128, EC], F32, isOutput=False)
    bb_in = nc.declare_dram_parameter("bb2d", [128, EC], F32, isOutput=False)
    ones_in = nc.declare_dram_parameter("ones_row", [1, 128], F32, isOutput=False)
    onesc_in = nc.declare_dram_parameter("ones_col", [128, 1], F32, isOutput=False)
    yT_out = nc.declare_dram_parameter("yT", [E, TOK], F32, isOutput=True)

    RG = [list(range(NCORES))]

    with tile.TileContext(nc) as tc:
        from contextlib import ExitStack
        with ExitStack() as ctx:
            dram = ctx.enter_context(tc.tile_pool(name="dram", bufs=1, space="DRAM"))
            pers = ctx.enter_context(tc.tile_pool(name="pers", bufs=1))
            ps_sc = ctx.enter_context(tc.tile_pool(name="ps_sc", bufs=2, space="PSUM"))
            ps_o = ctx.enter_context(tc.tile_pool(name="ps_o", bufs=1, space="PSUM"))
            ps_post = ctx.enter_context(tc.tile_pool(name="ps_post", bufs=2, space="PSUM"))

            # DRAM bounce + gathered buffers for the collectives
            k_bb = dram.tile([E, TOK], F32, name="k_bb")
            v_bb = dram.tile([TOK, H * 65], F32, name="v_bb")
            k_g = dram.tile([NCORES, E, TOK], F32, addr_space="Shared", name="k_g")
            v_g = dram.tile([NCORES, TOK, H * 65], F32, addr_space="Shared", name="v_g")

            # ---- persistent tiles ----
            onr = pers.tile([1, 128], F32R, tag="onr")
            onc = pers.tile([128, 1], F32R, tag="onc")
            nc.sync.dma_start(onr[:], ones_in[:].bitcast(F32R))
            nc.sync.dma_start(onc[:], onesc_in[:].bitcast(F32R))
            bo2d = pers.tile([128, EC], F32, tag="bo2d")
            b12d = pers.tile([128, HIDC], F32, tag="b12d")
            b22d = pers.tile([128, EC], F32, tag="b22d")
            g2d = pers.tile([128, EC], F32, tag="g2d")
            bb2d = pers.tile([128, EC], F32, tag="bb2d")

            xa = [pers.tile([128, TOK], F32R, tag=f"xa{i}", name=f"xa{i}") for i in range(EC)]
            qT = [pers.tile([128, TOK], F32R, tag=f"qT{i}", name=f"qT{i}") for i in range(EC)]
            woT = [pers.tile([128, E], F32R, tag=f"woT{i}", name=f"woT{i}") for i in range(EC)]
            w1T = [pers.tile([128, HID], F32R, tag=f"w1T{i}", name=f"w1T{i}") for i in range(EC)]
            w2T = [pers.tile([128, E], F32R, tag=f"w2T{i}", name=f"w2T{i}") for i in range(HIDC)]

            # ================= Stage Q: x+pe, QKV, AllGathers =================
            with tc.tile_pool(name="pq", bufs=1) as pq:
                peT = [pq.tile([128, TOK], F32, tag=f"peT{i}", name=f"peT{i}") for i in range(EC)]
                wqkT = [pq.tile([128, 2 * E], F32R, tag=f"wqkT{i}", name=f"wqkT{i}") for i in range(EC)]
                wvT = [pq.tile([128, E], F32R, tag=f"wvT{i}", name=f"wvT{i}") for i in range(EC)]
                bqk2d = pq.tile([128, 8], F32, tag="bqk2d")
                bv_rep = pq.tile([128, E], F32, tag="bv_rep")

                for i in range(EC):
                    nc.sync.dma_start(xa[i][:], xT_in[i * 128:(i + 1) * 128, :].bitcast(F32R))
                    nc.sync.dma_start(peT[i][:], peT_in[i * 128:(i + 1) * 128, :])
                    nc.sync.dma_start(wqkT[i][:], wqkT_in[i * 128:(i + 1) * 128, :].bitcast(F32R))
                    nc.sync.dma_start(wvT[i][:], wvT_in[i * 128:(i + 1) * 128, :].bitcast(F32R))
                nc.sync.dma_start(bqk2d[:], bqk_in[:])
                nc.sync.dma_start(bv_rep[:], bvr_in[:])

                for i in range(EC):
                    # x + pe, in place, rounded to f32r on write
                    nc.vector.tensor_add(xa[i][:], xa[i][:].bitcast(F32), peT[i][:])

                # K features first (m=4..7 of wqkT), so AG(k) can fire early
                def qk_feature(m, dest_tile, dest_dram):
                    psm = ps_sc.tile([128, TOK], F32, tag="sc", name="ps_qk")
                    for nch in range(2):
                        sl = slice(nch * 512, (nch + 1) * 512)
                        for k in range(EC):
                            nc.tensor.matmul(
                                psm[:, sl],
                                wqkT[k][:, m * 128:(m + 1) * 128],
                                xa[k][:, sl],
                                start=(k == 0), stop=(k == EC - 1),
                            )
                    nc.vector.tensor_scalar_add(dest_tile[:], psm[:], bqk2d[:, m:m + 1])
                    if dest_dram is not None:
                        nc.sync.dma_start(dest_dram, dest_tile[:])

                for m in range(4, 8):
                    kst = pq.tile([128, TOK], F32, tag="kst", bufs=2)
                    qk_feature(m, kst, k_bb[(m - 4) * 128:(m - 3) * 128, :])
                nc.gpsimd.collective_compute(
                    "AllGather", OP.bypass, replica_groups=RG,
                    ins=[k_bb.opt()], outs=[k_g.opt()],
                )
                for m in range(4):
                    qk_feature(m, qT[m], None)

                # V (token-major) + interleaved ones column, written per head block
                ones8 = pq.tile([128, 8], F32, tag="ones8")
                nc.vector.memset(ones8[:], 1.0)
                v_bb_r = v_bb.rearrange("t (h c) -> t h c", c=65)
                for mt in range(8):
                    psv = ps_post.tile([128, E], F32, tag="pp", name="ps_v")
                    for k in range(EC):
                        nc.tensor.matmul(
                            psv[:],
                            xa[k][:, mt * 128:(mt + 1) * 128],
                            wvT[k][:],
                            start=(k == 0), stop=(k == EC - 1),
                        )
                    vst = pq.tile([128, E], F32, tag="vst", bufs=2)
                    nc.vector.tensor_add(vst[:], psv[:], bv_rep[:])
                    rows = slice(mt * 128, (mt + 1) * 128)
                    for h in range(H):
                        nc.sync.dma_start(
                            v_bb_r[rows, h, 0:64], vst[:, h * 64:(h + 1) * 64]
                        )
                    nc.sync.dma_start(v_bb_r[rows, :, 64:65], ones8[:])
                nc.gpsimd.collective_compute(
                    "AllGather", OP.bypass, replica_groups=RG,
                    ins=[v_bb.opt()], outs=[v_g.opt()],
                )

            pa = ctx.enter_context(tc.tile_pool(name="pa", bufs=1))
            pp = ctx.enter_context(tc.tile_pool(name="pp", bufs=1))

            # FFN / out-proj weights + post biases (prefetch during the AGs)
            for i in range(EC):
                nc.sync.dma_start(woT[i][:], woT_in[i * 128:(i + 1) * 128, :].bitcast(F32R))
                nc.sync.dma_start(w1T[i][:], w1T_in[i * 128:(i + 1) * 128, :].bitcast(F32R))
            for i in range(HIDC):
                nc.sync.dma_start(w2T[i][:], w2T_in[i * 128:(i + 1) * 128, :].bitcast(F32R))
            nc.sync.dma_start(bo2d[:], bo_in[:])
            nc.sync.dma_start(b12d[:], b1_in[:])
            nc.sync.dma_start(b22d[:], b2_in[:])
            nc.sync.dma_start(g2d[:], g_in[:])
            nc.sync.dma_start(bb2d[:], bb_in[:])

            # ================= Stage A: attention for one (b, hg) ============
            _ATT = os.environ.get("K_ATT", "full")  # dma | sc | av | full
            _NOEXP = os.environ.get("K_NOEXP", "0") == "1"
            _NKT = int(os.environ.get("K_NKT", "100000"))

            def attention(b):
                oT = [pa.tile([128, QB], F32R, tag=f"oT{i}", bufs=2, name=f"oT{i}") for i in range(EC)]
                for hg in range(2):
                    pso = ps_o.tile([65, 4 * QB], F32, tag="ps_o")
                    for kt in range(16):
                        r, koff = kt // 2, (kt % 2) * 128
                        col0 = b * LC + koff
                        kA = pa.tile([128, 128], F32R, tag="ktA", bufs=4)
                        kB = pa.tile([128, 128], F32R, tag="ktB", bufs=4)
                        nc.sync.dma_start(
                            kA[:], k_g[r, hg * 256:hg * 256 + 128, col0:col0 + 128].bitcast(F32R))
                        nc.sync.dma_start(
                            kB[:], k_g[r, hg * 256 + 128:hg * 256 + 256, col0:col0 + 128].bitcast(F32R))
                        va = pa.tile([128, H * 65], F32R, tag="va", bufs=3)
                        nc.sync.dma_start(va[:], v_g[r, col0:col0 + 128, :].bitcast(F32R))

                        if _ATT == "dma" or (b * 2 + hg) * 16 + kt >= _NKT:
                            continue
                        pssc = ps_sc.tile([128, 4 * QB], F32, tag="sc", name="pssc")
                        # PE row-tiling: lhsT at partition 64 runs on tile T8,
                        # concurrent with T0 — their PSUM banks must be disjoint.
                        # T0 heads (even h4) -> slots 0,1 (bank0); T8 -> 2,3 (bank1).
                        SLOT = {0: 0, 2: 1, 1: 2, 3: 3}
                        for h4 in range(4):
                            h = hg * 4 + h4
                            ksrc = kA if h4 < 2 else kB
                            roff = (h4 % 2) * 64
                            sl = SLOT[h4]
                            nc.tensor.matmul(
                                pssc[:, sl * QB:(sl + 1) * QB],
                                ksrc[roff:roff + 64, :],
                                qT[h // 2][(h % 2) * 64:(h % 2) * 64 + 64, b * QB:(b + 1) * QB],
                                start=True, stop=True,
                            )
                        ex = pa.tile([128, 4 * QB], F32R, tag="ex", bufs=3)
                        if _NOEXP:
                            nc.vector.tensor_scalar_mul(ex[:], pssc[:], 0.125)
                        else:
                            nc.scalar.activation(ex[:], pssc[:], AF.Exp, scale=0.125)
                        if _ATT in ("av", "full"):
                            # start=True clears has_written for the WHOLE bank, so
                            # only the first group per bank (h4 even) may set it.
                            for h4 in range(4):
                                h = hg * 4 + h4
                                sl = SLOT[h4]
                                nc.tensor.matmul(
                                    pso[:, h4 * QB:(h4 + 1) * QB],
                                    va[:, h * 65:(h + 1) * 65],
                                    ex[:, sl * QB:(sl + 1) * QB],
                                    start=(kt == 0 and h4 % 2 == 0), stop=(kt == 15),
                                    skip_group_check=(h4 % 2 == 1),
                                )
                    if _ATT in ("dma", "sc", "av"):
                        continue
                    # normalize: r = 1/rowsum (psum row 64), replicate via K=1 matmul
                    rr = pa.tile([1, 4 * QB], F32R, tag="rr", bufs=1)
                    with nc.allow_low_precision(reason="softmax denom rounded to f32r"):
                        nc.vector.reciprocal(rr[:], pso[64:65, :])
                    for pair in range(2):
                        psr = ps_post.tile([128, 512], F32, tag="pp", name="ps_r")
                        for j in range(2):
                            h4 = pair * 2 + j
                            nc.tensor.matmul(
                                psr[0:64, j * QB:(j + 1) * QB],
                                onr[0:1, 0:64],
                                rr[0:1, h4 * QB:(h4 + 1) * QB],
                                start=True, stop=True,
                            )
                        rsb = pa.tile([64, 512], F32, tag="rsb", bufs=1)
                        nc.vector.tensor_copy(rsb[:], psr[0:64, :])
                        for j in range(2):
                            h4 = pair * 2 + j
                            h = hg * 4 + h4
                            nc.vector.tensor_mul(
                                oT[h // 2][(h % 2) * 64:(h % 2) * 64 + 64, :],
                                pso[0:64, h4 * QB:(h4 + 1) * QB],
                                rsb[:, j * QB:(j + 1) * QB],
                            )

                return oT

            # ================= Stage P: out-proj + LN1 + FFN + LN2 ===========
            def layer_norm(src_tiles, dst_tiles, dst_dtype_f32r):
                """src/dst: 4 tiles of (128, QB). dst written f32r (or f32 for output)."""
                pss = ps_post.tile([128, 512], F32, tag="pp", name="ps_ln")
                for k in range(EC):
                    nc.tensor.matmul(pss[0:1, 0:QB], onc[:], src_tiles[k][:],
                                     start=(k == 0), stop=(k == EC - 1))
                pss2 = ps_post.tile([128, 512], F32, tag="pp", name="ps_ln2")
                for k in range(EC):
                    sq = pp.tile([128, QB], F32R, tag="sq", bufs=2)
                    nc.vector.tensor_mul(sq[:], src_tiles[k][:].bitcast(F32), src_tiles[k][:].bitcast(F32))
                    nc.tensor.matmul(pss2[0:1, 0:QB], onc[:], sq[:],
                                     start=(k == 0), stop=(k == EC - 1))
                rows = pp.tile([1, 3 * QB], F32, tag="lnrows", bufs=1, name="lnrows")
                rowsr = pp.tile([1, 2 * QB], F32R, tag="lnrowsr", bufs=1, name="lnrowsr")
                mu = rows[0:1, 0:QB]
                mu2 = rows[0:1, QB:2 * QB]
                rec = rows[0:1, 2 * QB:3 * QB]
                mur = rowsr[0:1, 0:QB]
                rsq = rowsr[0:1, QB:2 * QB]
                nc.vector.tensor_scalar_mul(mu, pss[0:1, 0:QB], 1.0 / E)
                nc.vector.tensor_copy(mur, mu)
                nc.vector.tensor_mul(mu2, mu, mu)
                # var + eps, in place in mu2's slot successor: reuse rec slice as var
                nc.vector.scalar_tensor_tensor(rec, pss2[0:1, 0:QB], 1.0 / E, mu2,
                                               op0=OP.mult, op1=OP.subtract)
                nc.vector.tensor_scalar_add(rec, rec, 1e-5)
                nc.vector.reciprocal(rec, rec)
                nc.scalar.activation(rsq, rec, AF.Sqrt)
                psm = ps_post.tile([128, 512], F32, tag="pp", name="ps_rep")
                nc.tensor.matmul(psm[:, 0:QB], onr[:], mur, start=True, stop=True)
                nc.tensor.matmul(psm[:, QB:2 * QB], onr[:], rsq, start=True, stop=True)
                for k in range(EC):
                    t1 = pp.tile([128, QB], F32, tag="t1", bufs=2)
                    nc.vector.tensor_sub(t1[:], src_tiles[k][:].bitcast(F32), psm[:, 0:QB])
                    t2 = pp.tile([128, QB], F32, tag="t2", bufs=2)
                    nc.vector.tensor_mul(t2[:], t1[:], psm[:, QB:2 * QB])
                    nc.vector.tensor_scalar(dst_tiles[k][:], t2[:], g2d[:, k:k + 1],
                                            bb2d[:, k:k + 1], op0=OP.mult, op1=OP.add)

            def post(b, oT):
                xres = [pp.tile([128, QB], F32R, tag=f"xres{k}", name=f"xres{k}") for k in range(EC)]
                for m in range(EC):
                    pst = ps_post.tile([128, 512], F32, tag="pp", name="ps_op")
                    for k in range(EC):
                        nc.tensor.matmul(pst[:, 0:QB], woT[k][:, m * 128:(m + 1) * 128],
                                         oT[k][:], start=(k == 0), stop=(k == EC - 1))
                    nc.vector.scalar_tensor_tensor(
                        xres[m][:], pst[:, 0:QB], bo2d[:, m:m + 1],
                        xa[m][:, b * QB:(b + 1) * QB].bitcast(F32),
                        op0=OP.add, op1=OP.add)
                x1 = [pp.tile([128, QB], F32R, tag=f"x1{k}", name=f"x1{k}") for k in range(EC)]
                layer_norm(xres, x1, True)
                hT = [pp.tile([128, 512], F32R, tag=f"hT{i}", name=f"hT{i}") for i in range(8)]
                for mp in range(8):
                    for j in range(2):
                        m = mp * 2 + j
                        psf = ps_post.tile([128, 512], F32, tag="pp", name="ps_f1")
                        for k in range(EC):
                            nc.tensor.matmul(psf[:, 0:QB],
                                             w1T[k][:, m * 128:(m + 1) * 128],
                                             x1[k][:], start=(k == 0), stop=(k == EC - 1))
                        nc.scalar.activation(hT[mp][:, j * QB:(j + 1) * QB],
                                             psf[:, 0:QB],
                                             AF.Gelu, bias=b12d[:, m:m + 1])
                xf = [pp.tile([128, QB], F32R, tag=f"xf{k}", name=f"xf{k}") for k in range(EC)]
                for m2 in range(EC):
                    psg = ps_post.tile([128, 512], F32, tag="pp", name="ps_f2")
                    for k2 in range(HIDC):
                        nc.tensor.matmul(psg[:, 0:QB],
                                         w2T[k2][:, m2 * 128:(m2 + 1) * 128],
                                         hT[k2 // 2][:, (k2 % 2) * QB:(k2 % 2 + 1) * QB],
                                         start=(k2 == 0), stop=(k2 == HIDC - 1))
                    nc.vector.scalar_tensor_tensor(
                        xf[m2][:], psg[:, 0:QB], b22d[:, m2:m2 + 1], x1[m2][:].bitcast(F32),
                        op0=OP.add, op1=OP.add)
                _DUMP = os.environ.get("K_DUMP", "")
                if _DUMP == "x1":
                    if b == 3:
                        for m in range(EC):
                            nc.sync.dma_start(yT_out[m * 128:(m + 1) * 128, 0:QB], x1[m][:].bitcast(F32))
                elif _DUMP == "xres":
                    if b == 3:
                        for m in range(EC):
                            nc.sync.dma_start(yT_out[m * 128:(m + 1) * 128, 0:QB], xres[m][:].bitcast(F32))
                elif _DUMP == "xf":
                    if b == 3:
                        for m in range(EC):
                            nc.sync.dma_start(yT_out[m * 128:(m + 1) * 128, 0:QB], xf[m][:].bitcast(F32))
                else:
                    yt = [pp.tile([128, QB], F32, tag=f"yt{k}", name=f"yt{k}") for k in range(EC)]
                    layer_norm(xf, yt, False)
                    for m in range(EC):
                        nc.sync.dma_start(yT_out[m * 128:(m + 1) * 128, b * QB:(b + 1) * QB], yt[m][:])

            # software-pipelined emission: attn(b+1) before post(b)
            if _STAGE == "q":
                if os.environ.get("K_DUMPQ", "0") == "1":
                    for m in range(EC):
                        nc.sync.dma_start(yT_out[m * 128:(m + 1) * 128, :], qT[m][:].bitcast(F32))
            elif _STAGE == "qa":
                last = None
                for b in range(B):
                    last = attention(b)
                if _ATT == "full":
                    for m in range(EC):
                        nc.sync.dma_start(yT_out[m * 128:(m + 1) * 128, 0:QB], last[m][:].bitcast(F32))
            elif os.environ.get("K_NOPIPE", "0") == "1":
                for b in range(B):
                    post(b, attention(b))
            else:
                o0 = attention(0)
                o1 = attention(1)
                post(0, o0)
                o2 = attention(2)
                post(1, o1)
                o3 = attention(3)
                post(2, o2)
                post(3, o3)

    nc.compile()
    _BUILD_CACHE["nc"] = nc
    return nc


def _pos_encoding_np(S, Emb):
    t = np.arange(S, dtype=np.float32)[:, None]
    i = np.arange(Emb, dtype=np.float32)[None, :]
    even = np.sin((t + 1.0) * np.power(np.float32(10000.0), -i / Emb))
    odd = np.cos((t + 1.0) * np.power(np.float32(10000.0), -(i + 1.0) / Emb))
    return np.where(np.arange(Emb)[None, :] % 2 == 0, even, odd).astype(np.float32)


def prepare_in_maps(x, in_proj_w, in_proj_b, out_w, out_b, w1, b1, w2, b2, ln_g, ln_b):
    pe = _pos_encoding_np(B, E)                      # (B, E)
    peT = np.repeat(pe, LC, axis=0).T                # (E, TOK), same on every core
    shared = {
        "peT": np.ascontiguousarray(peT),
        "wqkT": np.ascontiguousarray(in_proj_w[:2 * E].T),
        "wvT": np.ascontiguousarray(in_proj_w[2 * E:].T),
        "woT": np.ascontiguousarray(out_w.T),
        "w1T": np.ascontiguousarray(w1.T),
        "w2T": np.ascontiguousarray(w2.T),
        "bqk2d": np.ascontiguousarray(in_proj_b[:2 * E].reshape(8, 128).T),
        "bv_rep": np.ascontiguousarray(np.tile(in_proj_b[2 * E:], (128, 1))),
        "bo2d": np.ascontiguousarray(out_b.reshape(EC, 128).T),
        "b1_2d": np.ascontiguousarray(b1.reshape(HIDC, 128).T),
        "b2_2d": np.ascontiguousarray(b2.reshape(EC, 128).T),
        "g2d": np.ascontiguousarray(ln_g.reshape(EC, 128).T),
        "bb2d": np.ascontiguousarray(ln_b.reshape(EC, 128).T),
        "ones_row": np.ones((1, 128), np.float32),
        "ones_col": np.ones((128, 1), np.float32),
    }
    in_maps = []
    for c in range(NCORES):
        xc = x[c * LC:(c + 1) * LC]                  # (LC, B, E)
        xc = np.transpose(xc, (1, 0, 2)).reshape(TOK, E)   # batch-major tokens
        m = dict(shared)
        m["xT"] = np.ascontiguousarray(xc.T)         # (E, TOK)
        in_maps.append(m)
    return in_maps


def assemble_output(results):
    y = np.empty((L, B, E), np.float32)
    for c in range(NCORES):
        yTc = results[c]["yT"]                       # (E, TOK)
        yc = yTc.T.reshape(B, LC, E)                 # batch-major back to (B, LC, E)
        y[c * LC:(c + 1) * LC] = np.transpose(yc, (1, 0, 2))
    return y


def kernel(**inputs):
    inputs = {k: np.asarray(v, dtype=np.float32) for k, v in inputs.items()}
    nc = build_encoder()
    in_maps = prepare_in_maps(**inputs)
    res = run_bass_kernel_spmd(nc, in_maps, core_ids=list(range(NCORES)))
    return assemble_output(res.results)
